# revision 24
# baseline (speedup 1.0000x reference)
"""KAN layer (B=8192, IN_F=OUT_F=1024, GRID=5) on 8 Trainium2 cores.

Math: Y[b,o] = W0[o]*silu(x) + spline_o(clip(x,-1,1)) * W1[o] + b[o], x = X[b,o]
(idx_in = arange(O) % IN_F is the identity here since O == IN_F).

Factorization used here (clip-form basis, exact):
  Y = W0*silu(x) + B'*xc + G1*M1 + G2*M2 + G3*M3 + A''
  xc  = clip(x, -1, 1)
  Mj  = clip(x, s_j, 1),  s_j in {-0.5, 0.0, 0.5}
  B'  = w1*sl0;  Gj = w1*(sl_j - sl_{j-1});  sl_g = 2*(c_{g+1}-c_g)
  A'' = w1*(c0 + sl0 + 0.5*d1 - 0.5*d3) + b   (d_j = sl_j - sl_{j-1})

Sharding: EDGES across the 8 cores (128 edges per core, full batch 8192 on
the free dim).  Per core only 5 diagonal stationaries are needed; X arrives
pre-transposed and cast to fp16 on host ([128 edges, 8192 batch]), output
returns as fp16 and is cast back on host.  This halves DMA traffic in both
directions (tolerance is 2e-2; fp16 I/O costs ~1e-3).

Per 512-col chunk (one PSUM bank) the per-edge weighted sum runs on
TensorE as diagonal fp16 matmuls accumulating in PSUM.  Chunks of the
AFF_GROUPS subgroups skip the M3 matmul: their evacuation runs on DVE as
affine_then_add (yo = M3*G3 + A'' + psum); other chunks evacuate on
ScalarE (Identity + per-edge A'' bias) after a 5th matmul.  silu on
ScalarE, xc/M1/M3 on DVE (tensor_scalar, 4x fp16 mode), M2 on GpSimd.
x loads + y stores ride the Sync (SP) HWDGE queue (inputs fully
prefetched, first chunks small); the tiny weight/identity loads ride the
ScalarE HWDGE queue.  Warmup ops (on private scratch, no hazards) open
the PE clock gate and preload both ACT table sets during the framework
preamble; the last subgroup splits its evacuation across both engines and
its stores in two for a short tail.
"""
import sys

for _p in ("/root/.axon_site", "/root/.axon_site/_ro/trn_rl_repo", "/root/.axon_site/_ro/pypackages"):
    if _p not in sys.path:
        sys.path.append(_p)

import numpy as np

import concourse.bacc as bacc
import concourse.tile as tile
from concourse import mybir
from concourse.bass_utils import run_bass_kernel_spmd

B, IN_F, OUT_F, GRID = 8192, 1024, 1024, 5
N_CORES = 8
E_SHARD = OUT_F // N_CORES      # 128 edges per core
NG = 8                          # batch groups of 1024
GW = B // NG                    # group width (1024)
IN_SPLIT = (1024, 1024, 2048, 2048, 2048)   # input DMA / feature widths
CHUNK = 512                     # one PSUM bank of fp32
N_WARM = 7                      # PE clock-gate warmup matmuls
M2_ON_POOL = True               # compute M2 on GpSimd (else DVE)
AFF_GROUPS = (0, 2, 4, 6)       # subgroups evacuated via DVE affine_then_add
XC_POOL_GROUPS = ()             # subgroups whose xc is computed on GpSimd

_nc_cache = None


def _build():
    f16 = mybir.dt.float16
    f32 = mybir.dt.float32
    AF = mybir.ActivationFunctionType
    OP = mybir.AluOpType
    nc = bacc.Bacc("TRN2", target_bir_lowering=False, debug=False)
    xt = nc.dram_tensor("xt", [E_SHARD, B], f16, kind="ExternalInput").ap()
    wp = nc.dram_tensor("wp", [E_SHARD, 8], f32, kind="ExternalInput").ap()
    idn = nc.dram_tensor("idn", [E_SHARD, E_SHARD], f16, kind="ExternalInput").ap()
    yt = nc.dram_tensor("yt", [E_SHARD, B], f16, kind="ExternalOutput").ap()

    with tile.TileContext(nc) as tc:
        with tc.tile_pool(name="const", bufs=1) as cpool, \
             tc.tile_pool(name="xin", bufs=NG) as xpool, \
             tc.tile_pool(name="fsil", bufs=3) as spool, \
             tc.tile_pool(name="fxc", bufs=3) as xcpool, \
             tc.tile_pool(name="fm1", bufs=3) as m1pool, \
             tc.tile_pool(name="fm2", bufs=3) as m2pool, \
             tc.tile_pool(name="fm3", bufs=3) as m3pool, \
             tc.tile_pool(name="yout", bufs=3) as ypool, \
             tc.tile_pool(name="ps", bufs=6, space="PSUM") as pspool, \
             tc.tile_pool(name="pswarm", bufs=1, space="PSUM") as wpool:

            # --- warmups: each op on its own scratch tile (no hazards) so
            # they all run during the framework preamble: PE clock gate
            # opens, both ACT table sets load, DVE/Pool wake up
            scr = cpool.tile([128, CHUNK], f16)
            dum = cpool.tile([128, 16], f16)

            # identity matrix first on the SP queue (smallest latency path
            # to the diag builds); weights first on the ACT HWDGE queue
            idt = cpool.tile([128, 128], f16)
            nc.sync.dma_start(idt[:], idn[:, :])
            wpt = cpool.tile([128, 8], f32)
            nc.scalar.dma_start(wpt[:], wp[:, :])

            nc.scalar.activation(dum[:, 0:1], dum[:, 0:1], AF.Silu)
            nc.scalar.activation(dum[:, 1:2], dum[:, 1:2], AF.Identity)
            for w in range(3):
                nc.vector.tensor_scalar(dum[:, 2 + w:3 + w], dum[:, 2 + w:3 + w],
                                        1.0, -1.0, OP.min, OP.max)
            nc.gpsimd.tensor_scalar(dum[:, 5:6], dum[:, 5:6],
                                    1.0, -1.0, OP.min, OP.max)
            nc.vector.memset(scr[:], 0.0)
            ps_warm = wpool.tile([128, GW], f32, tag="pswarm", name="pswarm")
            for _ in range(N_WARM):
                nc.tensor.matmul(ps_warm[:, 0:CHUNK], scr[:, 0:128], scr[:],
                                 start=True, stop=True, skip_group_check=True)

            # --- full input prefetch on the SP queue (first chunks small
            # so compute starts as soon as possible)
            xg = []      # (tile, first subgroup, #subgroups)
            col = 0
            for d, w in enumerate(IN_SPLIT):
                x_t = xpool.tile([128, w], f16, tag=f"x{d}", name=f"x{d}")
                nc.sync.dma_start(x_t[:], xt[:, col:col + w])
                xg.append((x_t, col // GW, w // GW))
                col += w

            # --- diagonal stationaries on DVE: diag[f] = idn * w_f
            # order: 0=W0(silu) 1=B'(xc) 2=G1(M1) 3=G2(M2) 4=G3(M3)
            diag = cpool.tile([128, 5, 128], f16)
            for f in range(5):
                nc.vector.tensor_scalar_mul(diag[:, f, :], idt[:], wpt[:, f:f + 1])
            g3c = wpt[:, 4:5]
            apc = wpt[:, 5:6]

            feats = {}      # subgroup -> feature tiles
            gmap = {}       # subgroup -> (tile index, offset within tile)
            for ti, (x_t, g0, ng) in enumerate(xg):
                for i in range(ng):
                    gmap[g0 + i] = (ti, i)

            def emit_features(g):
                ti, i = gmap[g]
                xv = xg[ti][0][:, i * GW:(i + 1) * GW]
                sil = spool.tile([128, GW], f16, tag="sil", name=f"sil{g}")
                nc.scalar.activation(sil[:], xv, AF.Silu)
                xc = xcpool.tile([128, GW], f16, tag="xc", name=f"xc{g}")
                if g in XC_POOL_GROUPS:
                    nc.gpsimd.tensor_scalar(xc[:], xv, 1.0, -1.0, OP.min, OP.max)
                else:
                    nc.vector.tensor_scalar(xc[:], xv, 1.0, -1.0, OP.min, OP.max)
                m1 = m1pool.tile([128, GW], f16, tag="m1", name=f"m1{g}")
                nc.vector.tensor_scalar(m1[:], xv, 1.0, -0.5, OP.min, OP.max)
                m2 = m2pool.tile([128, GW], f16, tag="m2", name=f"m2{g}")
                if M2_ON_POOL and g > 0:
                    nc.gpsimd.tensor_scalar(m2[:], xv, 1.0, 0.0, OP.min, OP.max)
                else:
                    nc.vector.tensor_scalar(m2[:], xv, 1.0, 0.0, OP.min, OP.max)
                m3 = m3pool.tile([128, GW], f16, tag="m3", name=f"m3{g}")
                nc.vector.tensor_scalar(m3[:], xv, 1.0, 0.5, OP.min, OP.max)
                feats[g] = (sil, xc, m1, m2, m3)

            yo_cur = [None]

            def emit_subgroup(g):
                """matmuls + evac + store for 1024-col subgroup g."""
                sil, xc, m1, m2, m3 = feats.pop(g)
                base = 0
                if g % 2 == 0:
                    yo_cur[0] = ypool.tile([128, 2 * GW], f16, tag="yo",
                                           name=f"yo{g // 2}")
                yo = yo_cur[0]
                yb = (g % 2) * GW
                last = (g == NG - 1)
                dve_evac = g in AFF_GROUPS
                ps_last = (wpool.tile([128, GW], f32, tag="pswarm", name="ps_last")
                           if last else None)
                for h in range(2):
                    cs = slice(base + h * CHUNK, base + (h + 1) * CHUNK)
                    if last:
                        pc = ps_last[:, h * CHUNK:(h + 1) * CHUNK]
                    else:
                        pc = pspool.tile([128, CHUNK], f32, tag="ps",
                                         name=f"ps{g}_{h}")[:]
                    # last subgroup: h0 evacs on DVE, h1 on ACT (concurrent
                    # short tail; identity is the faster final evac)
                    aff = (dve_evac and not last) or (last and h == 0)
                    nc.tensor.matmul(pc, diag[:, 1, :], xc[:, cs],
                                     start=True, stop=False, skip_group_check=True)
                    nc.tensor.matmul(pc, diag[:, 2, :], m1[:, cs],
                                     start=False, stop=False, skip_group_check=True)
                    nc.tensor.matmul(pc, diag[:, 3, :], m2[:, cs],
                                     start=False, stop=False, skip_group_check=True)
                    if not aff:
                        nc.tensor.matmul(pc, diag[:, 4, :], m3[:, cs],
                                         start=False, stop=False,
                                         skip_group_check=True)
                    nc.tensor.matmul(pc, diag[:, 0, :], sil[:, cs],
                                     start=False, stop=True, skip_group_check=True)
                    ye = slice(yb + h * CHUNK, yb + (h + 1) * CHUNK)
                    if aff:
                        nc.vector.affine_then_add(yo[:, ye], m3[:, cs], pc,
                                                  scale=g3c, bias=apc)
                    else:
                        nc.scalar.activation(yo[:, ye], pc, AF.Identity,
                                             bias=apc, scale=1.0)
                    if last:
                        nc.sync.dma_start(yt[:, g * GW + h * CHUNK:
                                             g * GW + (h + 1) * CHUNK],
                                          yo[:, ye])
                if not last:
                    if g % 2 == 1:
                        nc.sync.dma_start(yt[:, (g - 1) * GW:(g + 1) * GW], yo[:])
                    elif g == NG - 2:
                        # penultimate subgroup stores alone (last is split)
                        nc.sync.dma_start(yt[:, g * GW:(g + 1) * GW],
                                          yo[:, 0:GW])

            # software-pipelined emission: features run one subgroup ahead
            # of the matmul/evac stream so in-order queues never head-block
            emit_features(0)
            for g in range(1, NG):
                emit_features(g)
                emit_subgroup(g - 1)
            emit_subgroup(NG - 1)
    nc.compile()
    return nc


def _host_prep(X, coeffs, W, b):
    c = coeffs.astype(np.float64)
    Wd = W.astype(np.float64)
    bd = b.astype(np.float64)
    sl = 2.0 * (c[:, 1:] - c[:, :-1])           # [O, 4] segment slopes
    d = sl[:, 1:] - sl[:, :-1]                  # [O, 3] slope deltas at knots
    w1 = Wd[:, 1]
    bprime = w1 * sl[:, 0]
    g = w1[:, None] * d                         # [O, 3]
    app = w1 * (c[:, 0] + sl[:, 0] + 0.5 * d[:, 0] - 0.5 * d[:, 2]) + bd

    wp = np.zeros((OUT_F, 8), dtype=np.float32)
    wp[:, 0] = Wd[:, 0]
    wp[:, 1] = bprime
    wp[:, 2] = g[:, 0]
    wp[:, 3] = g[:, 1]
    wp[:, 4] = g[:, 2]
    wp[:, 5] = app
    idn = np.eye(E_SHARD, dtype=np.float16)
    return wp, idn


def _in_maps(X, coeffs, W, b):
    wp, idn = _host_prep(X, coeffs, W, b)
    in_maps = []
    for c in range(N_CORES):
        sl = slice(c * E_SHARD, (c + 1) * E_SHARD)
        xt = np.ascontiguousarray(X[:, sl].T.astype(np.float16))
        in_maps.append({"xt": xt, "wp": np.ascontiguousarray(wp[sl]), "idn": idn})
    return in_maps


def kernel(X, coeffs, W, b):
    global _nc_cache
    if _nc_cache is None:
        _nc_cache = _build()
    nc = _nc_cache

    in_maps = _in_maps(X, coeffs, W, b)
    res = run_bass_kernel_spmd(nc, in_maps, core_ids=list(range(N_CORES)))
    Y = np.empty((B, OUT_F), dtype=np.float32)
    for c in range(N_CORES):
        sl = slice(c * E_SHARD, (c + 1) * E_SHARD)
        Y[:, sl] = res.results[c]["yt"].T.astype(np.float32)
    return Y


# revision 25
# speedup vs baseline: 1.0301x; 1.0301x over previous
"""KAN layer (B=8192, IN_F=OUT_F=1024, GRID=5) on 8 Trainium2 cores.

Math: Y[b,o] = W0[o]*silu(x) + spline_o(clip(x,-1,1)) * W1[o] + b[o], x = X[b,o]
(idx_in = arange(O) % IN_F is the identity here since O == IN_F).

Factorization used here (clip-form basis, exact):
  Y = W0*silu(x) + B'*xc + G1*M1 + G2*M2 + G3*M3 + A''
  xc  = clip(x, -1, 1)
  Mj  = clip(x, s_j, 1),  s_j in {-0.5, 0.0, 0.5}
  B'  = w1*sl0;  Gj = w1*(sl_j - sl_{j-1});  sl_g = 2*(c_{g+1}-c_g)
  A'' = w1*(c0 + sl0 + 0.5*d1 - 0.5*d3) + b   (d_j = sl_j - sl_{j-1})

Sharding: EDGES across the 8 cores (128 edges per core, full batch 8192 on
the free dim).  Per core only 5 diagonal stationaries are needed; X arrives
pre-transposed and cast to fp16 on host ([128 edges, 8192 batch]), output
returns as fp16 and is cast back on host.  This halves DMA traffic in both
directions (tolerance is 2e-2; fp16 I/O costs ~1e-3).

Per 512-col chunk (one PSUM bank) the per-edge weighted sum runs on
TensorE as diagonal fp16 matmuls accumulating in PSUM.  Chunks of the
AFF_GROUPS subgroups skip the M3 matmul: their evacuation runs on DVE as
affine_then_add (yo = M3*G3 + A'' + psum); other chunks evacuate on
ScalarE (Identity + per-edge A'' bias) after a 5th matmul.  silu on
ScalarE, xc/M1/M3 on DVE (tensor_scalar, 4x fp16 mode), M2 on GpSimd.
x loads + y stores ride the Sync (SP) HWDGE queue (inputs fully
prefetched, first chunks small); the tiny weight/identity loads ride the
ScalarE HWDGE queue.  Warmup ops (on private scratch, no hazards) open
the PE clock gate and preload both ACT table sets during the framework
preamble; the last subgroup splits its evacuation across both engines and
its stores in two for a short tail.
"""
import sys

for _p in ("/root/.axon_site", "/root/.axon_site/_ro/trn_rl_repo", "/root/.axon_site/_ro/pypackages"):
    if _p not in sys.path:
        sys.path.append(_p)

import numpy as np

import concourse.bacc as bacc
import concourse.tile as tile
from concourse import mybir
from concourse.bass_utils import run_bass_kernel_spmd

B, IN_F, OUT_F, GRID = 8192, 1024, 1024, 5
N_CORES = 8
E_SHARD = OUT_F // N_CORES      # 128 edges per core
NG = 8                          # batch groups of 1024
GW = B // NG                    # group width (1024)
IN_SPLIT = (1024, 1024, 2048, 2048, 2048)   # input DMA / feature widths
CHUNK = 512                     # one PSUM bank of fp32
N_WARM = 7                      # PE clock-gate warmup matmuls
M2_ON_POOL = True               # compute M2 on GpSimd (else DVE)
AFF_GROUPS = (0, 2, 4, 6)       # subgroups evacuated via DVE affine_then_add
XC_POOL_GROUPS = ()             # subgroups whose xc is computed on GpSimd

_nc_cache = None


def _build():
    f16 = mybir.dt.float16
    f32 = mybir.dt.float32
    AF = mybir.ActivationFunctionType
    OP = mybir.AluOpType
    nc = bacc.Bacc("TRN2", target_bir_lowering=False, debug=False)
    xt = nc.dram_tensor("xt", [E_SHARD, B], f16, kind="ExternalInput").ap()
    wp = nc.dram_tensor("wp", [E_SHARD, 8], f32, kind="ExternalInput").ap()
    idn = nc.dram_tensor("idn", [E_SHARD, E_SHARD], f16, kind="ExternalInput").ap()
    yt = nc.dram_tensor("yt", [E_SHARD, B], f16, kind="ExternalOutput").ap()

    with tile.TileContext(nc) as tc:
        with tc.tile_pool(name="const", bufs=1) as cpool, \
             tc.tile_pool(name="xin", bufs=NG) as xpool, \
             tc.tile_pool(name="fsil", bufs=3) as spool, \
             tc.tile_pool(name="fxc", bufs=3) as xcpool, \
             tc.tile_pool(name="fm1", bufs=3) as m1pool, \
             tc.tile_pool(name="fm2", bufs=3) as m2pool, \
             tc.tile_pool(name="fm3", bufs=3) as m3pool, \
             tc.tile_pool(name="yout", bufs=3) as ypool, \
             tc.tile_pool(name="ps", bufs=6, space="PSUM") as pspool, \
             tc.tile_pool(name="pswarm", bufs=1, space="PSUM") as wpool:

            # --- warmups: each op on its own scratch tile (no hazards) so
            # they all run during the framework preamble: PE clock gate
            # opens, both ACT table sets load, DVE/Pool wake up
            scr = cpool.tile([128, CHUNK], f16)
            dum = cpool.tile([128, 16], f16)

            # identity matrix first on the SP queue (smallest latency path
            # to the diag builds); weights first on the ACT HWDGE queue
            idt = cpool.tile([128, 128], f16)
            nc.sync.dma_start(idt[:], idn[:, :])
            wpt = cpool.tile([128, 8], f32)
            nc.scalar.dma_start(wpt[:], wp[:, :])

            nc.scalar.activation(dum[:, 0:1], dum[:, 0:1], AF.Silu)
            nc.scalar.activation(dum[:, 1:2], dum[:, 1:2], AF.Identity)
            for w in range(3):
                nc.vector.tensor_scalar(dum[:, 2 + w:3 + w], dum[:, 2 + w:3 + w],
                                        1.0, -1.0, OP.min, OP.max)
            nc.gpsimd.tensor_scalar(dum[:, 5:6], dum[:, 5:6],
                                    1.0, -1.0, OP.min, OP.max)
            nc.vector.memset(scr[:], 0.0)
            ps_warm = wpool.tile([128, GW], f32, tag="pswarm", name="pswarm")
            for _ in range(N_WARM):
                nc.tensor.matmul(ps_warm[:, 0:CHUNK], scr[:, 0:128], scr[:],
                                 start=True, stop=True, skip_group_check=True)

            # --- full input prefetch on the SP queue (first chunks small
            # so compute starts as soon as possible)
            xg = []      # (tile, first subgroup, #subgroups)
            col = 0
            for d, w in enumerate(IN_SPLIT):
                x_t = xpool.tile([128, w], f16, tag=f"x{d}", name=f"x{d}")
                nc.sync.dma_start(x_t[:], xt[:, col:col + w])
                xg.append((x_t, col // GW, w // GW))
                col += w

            # --- diagonal stationaries on DVE: diag[f] = idn * w_f
            # order: 0=W0(silu) 1=B'(xc) 2=G1(M1) 3=G2(M2) 4=G3(M3)
            diag = cpool.tile([128, 5, 128], f16)
            for f in range(5):
                nc.vector.tensor_scalar_mul(diag[:, f, :], idt[:], wpt[:, f:f + 1])
            g3c = wpt[:, 4:5]
            apc = wpt[:, 5:6]

            feats = {}      # subgroup -> feature tiles
            gmap = {}       # subgroup -> (tile index, offset within tile)
            for ti, (x_t, g0, ng) in enumerate(xg):
                for i in range(ng):
                    gmap[g0 + i] = (ti, i)

            def emit_features(g):
                ti, i = gmap[g]
                xv = xg[ti][0][:, i * GW:(i + 1) * GW]
                sil = spool.tile([128, GW], f16, tag="sil", name=f"sil{g}")
                nc.scalar.activation(sil[:], xv, AF.Silu)
                xc = xcpool.tile([128, GW], f16, tag="xc", name=f"xc{g}")
                if g in XC_POOL_GROUPS:
                    nc.gpsimd.tensor_scalar(xc[:], xv, 1.0, -1.0, OP.min, OP.max)
                else:
                    nc.vector.tensor_scalar(xc[:], xv, 1.0, -1.0, OP.min, OP.max)
                m1 = m1pool.tile([128, GW], f16, tag="m1", name=f"m1{g}")
                nc.vector.tensor_scalar(m1[:], xv, 1.0, -0.5, OP.min, OP.max)
                m2 = m2pool.tile([128, GW], f16, tag="m2", name=f"m2{g}")
                if M2_ON_POOL and g > 0:
                    nc.gpsimd.tensor_scalar(m2[:], xv, 1.0, 0.0, OP.min, OP.max)
                else:
                    nc.vector.tensor_scalar(m2[:], xv, 1.0, 0.0, OP.min, OP.max)
                m3 = m3pool.tile([128, GW], f16, tag="m3", name=f"m3{g}")
                nc.vector.tensor_scalar(m3[:], xv, 1.0, 0.5, OP.min, OP.max)
                feats[g] = (sil, xc, m1, m2, m3)

            yo_cur = [None]

            def emit_subgroup(g):
                """matmuls + evac + store for 1024-col subgroup g."""
                sil, xc, m1, m2, m3 = feats.pop(g)
                base = 0
                if g % 2 == 0:
                    yo_cur[0] = ypool.tile([128, 2 * GW], f16, tag="yo",
                                           name=f"yo{g // 2}")
                yo = yo_cur[0]
                yb = (g % 2) * GW
                last = (g == NG - 1)
                dve_evac = g in AFF_GROUPS
                ps_last = (wpool.tile([128, GW], f32, tag="pswarm", name="ps_last")
                           if last else None)
                for h in range(2):
                    cs = slice(base + h * CHUNK, base + (h + 1) * CHUNK)
                    if last:
                        pc = ps_last[:, h * CHUNK:(h + 1) * CHUNK]
                    else:
                        pc = pspool.tile([128, CHUNK], f32, tag="ps",
                                         name=f"ps{g}_{h}")[:]
                    # last subgroup: h0 evacs on DVE, h1 on ACT (concurrent
                    # short tail; identity is the faster final evac)
                    aff = (dve_evac and not last) or (last and h == 0)
                    nc.tensor.matmul(pc, diag[:, 1, :], xc[:, cs],
                                     start=True, stop=False, skip_group_check=True)
                    nc.tensor.matmul(pc, diag[:, 2, :], m1[:, cs],
                                     start=False, stop=False, skip_group_check=True)
                    nc.tensor.matmul(pc, diag[:, 3, :], m2[:, cs],
                                     start=False, stop=False, skip_group_check=True)
                    if not aff:
                        nc.tensor.matmul(pc, diag[:, 4, :], m3[:, cs],
                                         start=False, stop=False,
                                         skip_group_check=True)
                    nc.tensor.matmul(pc, diag[:, 0, :], sil[:, cs],
                                     start=False, stop=True, skip_group_check=True)
                    ye = slice(yb + h * CHUNK, yb + (h + 1) * CHUNK)
                    if aff:
                        nc.vector.affine_then_add(yo[:, ye], m3[:, cs], pc,
                                                  scale=g3c, bias=apc)
                    else:
                        nc.scalar.activation(yo[:, ye], pc, AF.Identity,
                                             bias=apc, scale=1.0)
                    if last:
                        nc.sync.dma_start(yt[:, g * GW + h * CHUNK:
                                             g * GW + (h + 1) * CHUNK],
                                          yo[:, ye])
                if not last:
                    if g % 2 == 1:
                        nc.sync.dma_start(yt[:, (g - 1) * GW:(g + 1) * GW], yo[:])
                    elif g == NG - 2:
                        # penultimate subgroup stores alone (last is split)
                        nc.sync.dma_start(yt[:, g * GW:(g + 1) * GW],
                                          yo[:, 0:GW])

            # software-pipelined emission: features run two subgroups ahead
            # of the matmul/evac stream so a slow feature op never stalls PE
            emit_features(0)
            emit_features(1)
            for g in range(2, NG):
                emit_features(g)
                emit_subgroup(g - 2)
            emit_subgroup(NG - 2)
            emit_subgroup(NG - 1)
    nc.compile()
    return nc


def _host_prep(X, coeffs, W, b):
    c = coeffs.astype(np.float64)
    Wd = W.astype(np.float64)
    bd = b.astype(np.float64)
    sl = 2.0 * (c[:, 1:] - c[:, :-1])           # [O, 4] segment slopes
    d = sl[:, 1:] - sl[:, :-1]                  # [O, 3] slope deltas at knots
    w1 = Wd[:, 1]
    bprime = w1 * sl[:, 0]
    g = w1[:, None] * d                         # [O, 3]
    app = w1 * (c[:, 0] + sl[:, 0] + 0.5 * d[:, 0] - 0.5 * d[:, 2]) + bd

    wp = np.zeros((OUT_F, 8), dtype=np.float32)
    wp[:, 0] = Wd[:, 0]
    wp[:, 1] = bprime
    wp[:, 2] = g[:, 0]
    wp[:, 3] = g[:, 1]
    wp[:, 4] = g[:, 2]
    wp[:, 5] = app
    idn = np.eye(E_SHARD, dtype=np.float16)
    return wp, idn


def _in_maps(X, coeffs, W, b):
    wp, idn = _host_prep(X, coeffs, W, b)
    in_maps = []
    for c in range(N_CORES):
        sl = slice(c * E_SHARD, (c + 1) * E_SHARD)
        xt = np.ascontiguousarray(X[:, sl].T.astype(np.float16))
        in_maps.append({"xt": xt, "wp": np.ascontiguousarray(wp[sl]), "idn": idn})
    return in_maps


def kernel(X, coeffs, W, b):
    global _nc_cache
    if _nc_cache is None:
        _nc_cache = _build()
    nc = _nc_cache

    in_maps = _in_maps(X, coeffs, W, b)
    res = run_bass_kernel_spmd(nc, in_maps, core_ids=list(range(N_CORES)))
    Y = np.empty((B, OUT_F), dtype=np.float32)
    for c in range(N_CORES):
        sl = slice(c * E_SHARD, (c + 1) * E_SHARD)
        Y[:, sl] = res.results[c]["yt"].T.astype(np.float32)
    return Y
